# revision 1
# baseline (speedup 1.0000x reference)
"""Multi-head attention on 8 Trainium2 NeuronCores (Bass/Tile).

Problem: B=4, T=2048, DIM=2048, H=16 heads, dk=dv=64.
  q = Q@Wq, k = K@Wk, v = V@Wv  (per head slices)
  out = softmax(q k^T / sqrt(dk)) v @ Wo

Sharding: data-parallel over batch (4) x query-row halves (2) = 8 cores.
Core (b, s) computes output rows [s*1024:(s+1)*1024] of batch b.
Each core projects k/v for its OWN T-half; the pair exchanges projected
k/v via a 2-rank AllGather (SCHEME_C). Attention + output projection are
core-local. With SCHEME_C=False each core recomputes the partner's k/v
projections instead (no collective).

Device layouts (bf16 compute, fp32 PSUM accumulation):
  xqT/xkT/xvT [D, TQ] = host-transposed input halves (D = contraction dim
    on partitions), wq/wk/wv [D, QK], wo [QK, D] natural (lhsT-ready)
  kT [QK, T]: head h rows 64h..64h+63 -> S^T matmul lhsT
  vaug [T, H, 65]: per head 64 v-cols + ones column (-> softmax row sums)
  S^T tile [Tk-chunk 128, Tq 512] = kT-chunk.T @ qT block (K=dk=64)
  P^T = exp(S^T/8)  (scores bounded ~+-5 -> no max-subtraction pass)
  aoT_aug [65, Tq] per head = vaug.T @ P^T accumulated over Tk chunks;
    row 64 = denominators l; rows/l via DRAM-bounce broadcast of 1/l
  out rows = aoT.T @ Wo accumulated over QK chunks.
"""

import os

import ml_dtypes
import numpy as np

import concourse.bass as bass
from concourse import bacc
import concourse.mybir as mybir
import concourse.tile as tile
from concourse.bass_utils import run_bass_kernel_spmd

BF16 = ml_dtypes.bfloat16
BF = mybir.dt.bfloat16
FP32 = mybir.dt.float32

B = 4
T = 2048
D = 2048
H = 16
DKH = 64
QK = H * DKH   # 1024
TQ = T // 2    # per-core query rows / local T half
KD = D // 128  # 16 contraction chunks for projections
NCORES = 8
SCHEME_C = False  # 2-rank AllGather measured slower than recomputing partner k/v
# SCHEME_R: each core projects k/v for its own T-half only and pushes the
# projected tiles straight into its pair partner's SBUF with relative-dest
# remote DMA (Δtpb=1 XOR). Softmax is permutation-invariant over keys, so
# kT/vaug store own half in chunks 0-7 and the partner half in 8-15 with no
# global-T reordering; attention code is unchanged.
SCHEME_R = False  # remote-DMA NEFF hangs at execution under the axon runtime
RSEM_EXPECT = 48  # 24 sends x (16 lanes / 8 dest slots) rsem increments

KN = 128 * (QK // 128) * TQ   # bf16 elems of one kT half
VN = 128 * (TQ // 128) * QK   # bf16 elems of one v half
CCN = KN + VN

LAST = None  # BassKernelResults of the most recent run (for test harness)

_cache = {}


def _install_ntff_shim():
    """Provide antenv.axon_hooks + disable artifact upload so that
    run_bass_kernel_spmd(trace=True) can profile under axon in this image."""
    import sys
    import types

    try:
        import antenv.axon_hooks  # noqa: F401
    except ImportError:
        import antenv
        mod = types.ModuleType("antenv.axon_hooks")
        _h = [None]
        mod.set_axon_ntff_profile_hook = lambda h: _h.__setitem__(0, h)
        mod.get_axon_ntff_profile_hook = lambda: _h[0]
        sys.modules["antenv.axon_hooks"] = mod
        antenv.axon_hooks = mod
        try:
            from trn_agent_boot.trn_boot import _ntff_profile_via_ctypes
            mod.set_axon_ntff_profile_hook(
                _ntff_profile_via_ctypes("/opt/axon/libaxon_pjrt.so"))
        except Exception as e:
            print(f"ntff hook registration failed: {e}")
    try:
        import concourse.bass_utils as bu
        bu.upload_artifacts = lambda tmpdir: f"local:{tmpdir}"
    except Exception:
        pass


def _emit(tc, xqT, xkT, xvT, wq, wk, wv, wo, out, cc_in, cc_out):
    nc = tc.nc
    exp_f = mybir.ActivationFunctionType.Exp
    n_halves = 1 if (SCHEME_C or SCHEME_R) else 2
    rdests = [(0, 1)] + [None] * 7
    rsem = lsem = bsem = None
    if SCHEME_R:
        rsem = nc.alloc_semaphore(name="kvx_r")
        lsem = nc.alloc_semaphore(name="kvx_l")
        bsem = nc.alloc_semaphore(name="kvx_b")

    with tc.tile_pool(name="persist", bufs=1) as persist:
        kT = persist.tile([128, QK // 128, T], BF, tag="kT")
        vaug = persist.tile([128, T // 128, H, DKH + 1], BF, tag="vaug")
        qT = persist.tile([128, QK // 128, TQ], BF, tag="qT")
        aoT = persist.tile([128, QK // 128, TQ], BF, tag="aoT")
        nc.vector.memset(vaug[:, :, :, DKH:DKH + 1], 1.0)
        if SCHEME_R:
            # sems are not cleared by allocation and persist across NEFF runs
            nc.gpsimd.sem_clear(rsem)
            nc.gpsimd.sem_clear(lsem)
            nc.gpsimd.sem_clear(bsem)

        # ---- phase 1: k/v projections (local half under SCHEME_C) ----
        with (
            nc.named_scope("p1_kvproj"),
            tc.tile_pool(name="wkv", bufs=1) as wkv_pool,
            tc.tile_pool(name="xk", bufs=17) as xk_pool,
            tc.tile_pool(name="xv", bufs=17) as xv_pool,
            tc.tile_pool(name="stg", bufs=6) as stg_pool,
            tc.tile_pool(name="ps1", bufs=6, space="PSUM") as ps1,
        ):
            wk_sb = wkv_pool.tile([128, KD, QK], BF, tag="wk")
            wv_sb = wkv_pool.tile([128, KD, QK], BF, tag="wv")
            ccin_f = cc_in[0, :] if SCHEME_C else None

            for nb in range(n_halves * TQ // 512):
                xk_t = []
                xv_t = []
                for k in range(KD):
                    xkt = xk_pool.tile([128, 512], BF, tag="xk")
                    xvt = xv_pool.tile([128, 512], BF, tag="xv")
                    # interleave weight-chunk and x-chunk loads so the first
                    # matmuls' inputs arrive first
                    if nb == 0:
                        nc.sync.dma_start(out=wk_sb[:, k, :], in_=wk[k * 128:(k + 1) * 128, :])
                        nc.sync.dma_start(out=wv_sb[:, k, :], in_=wv[k * 128:(k + 1) * 128, :])
                    nc.sync.dma_start(out=xkt, in_=xkT[k * 128:(k + 1) * 128, nb * 512:(nb + 1) * 512])
                    nc.sync.dma_start(out=xvt, in_=xvT[k * 128:(k + 1) * 128, nb * 512:(nb + 1) * 512])
                    xk_t.append(xkt)
                    xv_t.append(xvt)
                # kT[m-slice, this T block] = wk_slice.T @ xk
                for m in range(QK // 128):  # 8
                    ps = ps1.tile([128, 512], FP32, tag="ps1")
                    for k in range(KD):
                        nc.tensor.matmul(
                            ps, wk_sb[:, k, m * 128:(m + 1) * 128], xk_t[k],
                            start=(k == 0), stop=(k == KD - 1))
                    if SCHEME_C:
                        st = stg_pool.tile([128, 512], BF, tag="stg")
                        nc.vector.tensor_copy(out=st, in_=ps)
                        dst = bass.AP(
                            tensor=ccin_f.tensor,
                            offset=m * 1024 + nb * 512,
                            ap=[[QK // 128 * TQ, 128], [1, 512]])
                        nc.sync.dma_start(out=dst, in_=st)
                    else:
                        nc.vector.tensor_copy(out=kT[:, m, nb * 512:(nb + 1) * 512], in_=ps)
                        if SCHEME_R:
                            nc.gpsimd.remote_dma_broadcast(
                                out_ap=kT[:, m, TQ + nb * 512:TQ + (nb + 1) * 512],
                                in_ap=kT[:, m, nb * 512:(nb + 1) * 512],
                                remote_sem=rsem, local_sem=lsem, rdests=rdests)
                            nc.gpsimd.trigger_dma(count=None)
                            nc.gpsimd.inc_swdge_sem([bsem], [1], mode="add")
                # v[T-row slice, V cols] = xv_slice.T @ wv
                for msl in range(4):
                    ms = nb * 4 + msl
                    for n in range(QK // 512):  # 2
                        ps = ps1.tile([128, 512], FP32, tag="ps1")
                        for k in range(KD):
                            nc.tensor.matmul(
                                ps, xv_t[k][:, msl * 128:(msl + 1) * 128],
                                wv_sb[:, k, n * 512:(n + 1) * 512],
                                start=(k == 0), stop=(k == KD - 1))
                        if SCHEME_C:
                            st = stg_pool.tile([128, 512], BF, tag="stg")
                            nc.vector.tensor_copy(out=st, in_=ps)
                            dst = bass.AP(
                                tensor=ccin_f.tensor,
                                offset=KN + ms * 1024 + n * 512,
                                ap=[[TQ // 128 * QK, 128], [1, 512]])
                            nc.sync.dma_start(out=dst, in_=st)
                        else:
                            nc.vector.tensor_copy(
                                out=vaug[:, ms, n * 8:(n + 1) * 8, 0:DKH],
                                in_=ps.rearrange("p (h d) -> p h d", d=DKH))
                    if SCHEME_R:
                        nc.gpsimd.remote_dma_broadcast(
                            out_ap=vaug[:, 8 + ms, :, :],
                            in_ap=vaug[:, ms, :, :],
                            remote_sem=rsem, local_sem=lsem, rdests=rdests)
                        nc.gpsimd.trigger_dma(count=None)
                        nc.gpsimd.inc_swdge_sem([bsem], [1], mode="add")

                if nb == n_halves * TQ // 512 - 1:
                    # keep the PE array busy across the phase transition:
                    # a >3.4us idle gap lets the HAM re-throttle the clock
                    # to 1.2GHz for the next ~30us window. These filler
                    # matmuls read the last x tiles (so they schedule at
                    # the phase tail) and their results are never read.
                    for i in range(24):
                        ps = ps1.tile([128, 512], FP32, tag="ps1")
                        nc.tensor.matmul(
                            ps, xv_t[i % KD][:, 0:128], xk_t[(i + 1) % KD],
                            start=True, stop=True)

            if SCHEME_C:
                nc.gpsimd.collective_compute(
                    "AllGather", mybir.AluOpType.bypass,
                    replica_groups=[[0, 1], [2, 3], [4, 5], [6, 7]],
                    ins=[cc_in], outs=[cc_out])

        # ---- phase 2: q projection + attention ----
        with (
            nc.named_scope("p2_attn"),
            tc.tile_pool(name="wqp", bufs=1) as wq_pool,
            tc.tile_pool(name="xq", bufs=1) as xq_pool,
            tc.tile_pool(name="pt", bufs=12) as pt_pool,
            tc.tile_pool(name="dv", bufs=3) as dv_pool,
            tc.tile_pool(name="dsc", bufs=4, space="DRAM") as dr_pool,
            tc.tile_pool(name="psq", bufs=2, space="PSUM") as ps_q,
            tc.tile_pool(name="pss", bufs=2, space="PSUM") as ps_s,
            tc.tile_pool(name="pav", bufs=2, space="PSUM") as ps_av,
        ):
            wq_sb = wq_pool.tile([128, KD, QK], BF, tag="wq")
            xq_sb = xq_pool.tile([128, KD, TQ], BF, tag="xq")
            for k in range(KD):
                nc.sync.dma_start(out=wq_sb[:, k, :], in_=wq[k * 128:(k + 1) * 128, :])
                nc.sync.dma_start(out=xq_sb[:, k, :], in_=xqT[k * 128:(k + 1) * 128, :])

            if SCHEME_C:
                # scatter the gathered halves (rank order = T order) into
                # kT and vaug; overlaps with the q-projection below
                for r in range(2):
                    src_k = bass.AP(
                        tensor=cc_out.tensor,
                        offset=r * CCN,
                        ap=[[QK // 128 * TQ, 128], [TQ, QK // 128], [1, TQ]])
                    nc.sync.dma_start(out=kT[:, :, r * TQ:(r + 1) * TQ], in_=src_k)
                    for msl in range(TQ // 128):  # 8
                        ms = r * (TQ // 128) + msl
                        src_v = bass.AP(
                            tensor=cc_out.tensor,
                            offset=r * CCN + KN + msl * QK,
                            ap=[[TQ // 128 * QK, 128], [DKH, H], [1, DKH]])
                        nc.sync.dma_start(out=vaug[:, ms, :, 0:DKH], in_=src_v)

            for g in range(QK // 128):  # 8 head pairs
                for n in range(TQ // 512):  # 2
                    ps = ps_q.tile([128, 512], FP32, tag="psq")
                    for k in range(KD):
                        nc.tensor.matmul(
                            ps, wq_sb[:, k, g * 128:(g + 1) * 128],
                            xq_sb[:, k, n * 512:(n + 1) * 512],
                            start=(k == 0), stop=(k == KD - 1))
                    nc.vector.tensor_copy(out=qT[:, g, n * 512:(n + 1) * 512], in_=ps)

                if SCHEME_R and g == 0:
                    # Barrier by symmetry: bsem=24 means all my sends were
                    # handed to the SDMA engines; the partner (same program,
                    # lockstep NEFF start) is at the same point, and its
                    # earliest-written chunks are read first, its last chunk
                    # ~9us after this wait clears. The remote sem cannot be
                    # waited on directly: the scheduler's single-core sim
                    # never sees RDMA increments and declares deadlock.
                    nc.tensor.wait_ge(bsem, 24)

                for hp in range(2):
                    h = 2 * g + hp
                    pk = slice(hp * 64, (hp + 1) * 64)
                    for tqb in range(TQ // 512):  # 2
                        qs = qT[pk, g, tqb * 512:(tqb + 1) * 512]
                        pts = []
                        for t in range(8):  # pairs of Tk chunks
                            pss = ps_s.tile([128, 1024], FP32, tag="pss")
                            for c2 in range(2):
                                c = 2 * t + c2
                                nc.tensor.matmul(
                                    pss[:, c2 * 512:(c2 + 1) * 512],
                                    kT[pk, g, c * 128:(c + 1) * 128],
                                    qs, start=True, stop=True)
                            ptt = pt_pool.tile([128, 1024], BF, tag="pt")
                            nc.scalar.activation(out=ptt, in_=pss, func=exp_f, scale=0.125)
                            pts.append(ptt)
                        pav = ps_av.tile([DKH + 1, 512], FP32, tag="pav")
                        for c in range(T // 128):  # 16
                            nc.tensor.matmul(
                                pav, vaug[:, c, h, :],
                                pts[c // 2][:, (c % 2) * 512:(c % 2 + 1) * 512],
                                start=(c == 0), stop=(c == T // 128 - 1))
                        linv = dv_pool.tile([1, 512], FP32, tag="linv")
                        nc.vector.reciprocal(out=linv, in_=pav[DKH:DKH + 1, :])
                        ldr = dr_pool.tile([1, 512], FP32, tag="ldr")
                        nc.gpsimd.dma_start(out=ldr, in_=linv)
                        lbc = dv_pool.tile([DKH, 512], FP32, tag="lbc")
                        nc.gpsimd.dma_start(out=lbc, in_=ldr.to_broadcast([DKH, 512]))
                        # pre-copy on DVE so the 2-input mul carries only a
                        # DVE-local wait (TensorTensor ISA allows one wait)
                        lbcc = dv_pool.tile([DKH, 512], FP32, tag="lbcc")
                        nc.vector.tensor_copy(out=lbcc, in_=lbc)
                        att = dv_pool.tile([DKH, 512], BF, tag="att")
                        nc.vector.tensor_mul(out=att, in0=pav[0:DKH, :], in1=lbcc)
                        nc.sync.dma_start(
                            out=aoT[pk, g, tqb * 512:(tqb + 1) * 512], in_=att)

        # ---- phase 3: output projection ----
        with (
            nc.named_scope("p3_oproj"),
            tc.tile_pool(name="wo", bufs=32) as wo_pool,
            tc.tile_pool(name="ostg", bufs=6) as o_pool,
            tc.tile_pool(name="pso", bufs=6, space="PSUM") as ps_o,
        ):
            KO = QK // 128  # 8
            # all wo tiles upfront: no p2 dependency, so these DMAs land
            # during the attention tail and the first chains never wait
            wo_t = {}
            for nb in range(D // 512):  # 4
                for k in range(KO):
                    wot = wo_pool.tile([128, 512], BF, tag="wo")
                    nc.sync.dma_start(out=wot, in_=wo[k * 128:(k + 1) * 128, nb * 512:(nb + 1) * 512])
                    wo_t[(nb, k)] = wot
            # PE warmth bridge for the p2->p3 transition (see phase 1 tail):
            # reads aoT slice 6 (written near the end of attention) so the
            # scheduler places these in the gap before the first real MMs.
            for i in range(20):
                ps = ps_o.tile([128, 512], FP32, tag="pso")
                nc.tensor.matmul(
                    ps, aoT[:, 6, i * 128 % TQ:(i * 128 % TQ) + 128],
                    aoT[:, 6, 0:512], start=True, stop=True)
            for nb in range(D // 512):  # 4
                for m in range(TQ // 128):  # 8
                    ps = ps_o.tile([128, 512], FP32, tag="pso")
                    for k in range(KO):
                        nc.tensor.matmul(
                            ps, aoT[:, k, m * 128:(m + 1) * 128], wo_t[(nb, k)],
                            start=(k == 0), stop=(k == KO - 1))
                    stg = o_pool.tile([128, 512], FP32, tag="ostg")
                    nc.vector.tensor_copy(out=stg, in_=ps)
                    nc.sync.dma_start(
                        out=out[m * 128:(m + 1) * 128, nb * 512:(nb + 1) * 512], in_=stg)


def _build():
    if "nc" in _cache:
        return _cache["nc"]
    nc = bacc.Bacc("TRN2", target_bir_lowering=False, debug=False, num_devices=NCORES)
    xhalf = TQ if (SCHEME_C or SCHEME_R) else T
    xqT = nc.dram_tensor("xqT", [D, TQ], BF, kind="ExternalInput").ap()
    xkT = nc.dram_tensor("xkT", [D, xhalf], BF, kind="ExternalInput").ap()
    xvT = nc.dram_tensor("xvT", [D, xhalf], BF, kind="ExternalInput").ap()
    wq = nc.dram_tensor("wq", [D, QK], BF, kind="ExternalInput").ap()
    wk = nc.dram_tensor("wk", [D, QK], BF, kind="ExternalInput").ap()
    wv = nc.dram_tensor("wv", [D, QK], BF, kind="ExternalInput").ap()
    wo = nc.dram_tensor("wo", [QK, D], BF, kind="ExternalInput").ap()
    out = nc.dram_tensor("out", [TQ, D], mybir.dt.float32, kind="ExternalOutput").ap()
    cc_in = cc_out = None
    if SCHEME_C:
        cc_in = nc.dram_tensor("cc_in", [1, CCN], BF, kind="Internal").ap()
        cc_out = nc.dram_tensor("cc_out", [2, CCN], BF, kind="Internal").ap()
    with tile.TileContext(nc) as tc:
        _emit(tc, xqT, xkT, xvT, wq, wk, wv, wo, out, cc_in, cc_out)
    nc.compile()
    _cache["nc"] = nc
    return nc


def kernel(**inputs):
    global LAST
    Q = np.asarray(inputs["Q"], dtype=np.float32)
    K = np.asarray(inputs["K"], dtype=np.float32)
    V = np.asarray(inputs["V"], dtype=np.float32)
    wq_b = np.asarray(inputs["Wq"], dtype=np.float32).astype(BF16)
    wk_b = np.asarray(inputs["Wk"], dtype=np.float32).astype(BF16)
    wv_b = np.asarray(inputs["Wv"], dtype=np.float32).astype(BF16)
    wo_b = np.asarray(inputs["Wo"], dtype=np.float32).astype(BF16)

    nc = _build()
    in_maps = []
    for core in range(NCORES):
        b, s = core // 2, core % 2
        if SCHEME_C or SCHEME_R:
            xk = np.ascontiguousarray(K[b, s * TQ:(s + 1) * TQ, :].T).astype(BF16)
            xv = np.ascontiguousarray(V[b, s * TQ:(s + 1) * TQ, :].T).astype(BF16)
        else:
            xk = np.ascontiguousarray(K[b].T).astype(BF16)
            xv = np.ascontiguousarray(V[b].T).astype(BF16)
        in_maps.append({
            "xqT": np.ascontiguousarray(Q[b, s * TQ:(s + 1) * TQ, :].T).astype(BF16),
            "xkT": xk,
            "xvT": xv,
            "wq": wq_b, "wk": wk_b, "wv": wv_b, "wo": wo_b,
        })
    want_trace = bool(os.environ.get("BASS_TRACE"))
    if want_trace:
        _install_ntff_shim()
        try:
            res = run_bass_kernel_spmd(
                nc, in_maps, core_ids=list(range(NCORES)), trace=True)
        except Exception as e:  # profiling infra missing -> still get results
            print(f"trace run failed ({type(e).__name__}: {e}); retrying untraced")
            res = run_bass_kernel_spmd(nc, in_maps, core_ids=list(range(NCORES)))
    else:
        res = run_bass_kernel_spmd(nc, in_maps, core_ids=list(range(NCORES)))
    LAST = res
    if res.exec_time_ns is not None:
        print(f"HW exec time: {res.exec_time_ns} ns")

    out = np.empty((B, T, D), np.float32)
    for core in range(NCORES):
        b, s = core // 2, core % 2
        out[b, s * TQ:(s + 1) * TQ, :] = res.results[core]["out"]
    return out



# revision 14
# speedup vs baseline: 1.0105x; 1.0105x over previous
"""Multi-head attention on 8 Trainium2 NeuronCores (Bass/Tile).

Problem: B=4, T=2048, DIM=2048, H=16 heads, dk=dv=64.
  q = Q@Wq, k = K@Wk, v = V@Wv  (per head slices)
  out = softmax(q k^T / sqrt(dk)) v @ Wo

Sharding: data-parallel over batch (4) x query-row halves (2) = 8 cores.
Core (b, s) computes output rows [s*1024:(s+1)*1024] of batch b.
Each core projects k/v for its OWN T-half only; the pair exchanges the
projected tiles with two 2-rank AllGathers (one for k, one for v) that
overlap with the q projection. Attention + output projection are local.

Device layouts (bf16 compute, fp32 PSUM accumulation):
  xqT/xkT/xvT [D, TQ] = host-transposed input halves (D = contraction dim
    on partitions), wq/wk/wv [D, QK], wo [QK, D] natural (lhsT-ready)
  kT [QK, T]: head h rows 64h..64h+63 -> S^T matmul lhsT
  vaug [T, H, 65]: per head 64 v-cols + ones column (-> softmax row sums)
  Scores per head pair g: S^T chunk [128, 1024] computed as TWO row-tiled
    matmuls (K=dk=64): head 2g in PE row-tile (0,0) -> cols 0:512, head
    2g+1 in row-tile (64,0) -> cols 512:1024. Adjacent instructions on
    disjoint row groups execute concurrently in the PE array.
  P^T = exp(S^T/8)  (scores bounded ~+-5 -> no max-subtraction pass)
  pav [65, TQ-block] per head = vaug.T @ P^T accumulated over key chunks;
    row 64 = denominators l; rows/l via reciprocal_approx_fast +
    DRAM-bounce broadcast of 1/l for both heads at once
  out rows = aoT.T @ Wo accumulated over QK chunks.
"""

import os

import ml_dtypes
import numpy as np

import concourse.bass as bass
from concourse import bacc
import concourse.mybir as mybir
import concourse.tile as tile
from concourse.bass_utils import run_bass_kernel_spmd

BF16 = ml_dtypes.bfloat16
BF = mybir.dt.bfloat16
FP32 = mybir.dt.float32

B = 4
T = 2048
D = 2048
H = 16
DKH = 64
QK = H * DKH   # 1024
TQ = T // 2    # per-core query rows / local T half
KD = D // 128  # 16 contraction chunks for projections
NCORES = 8
GROUPS = [[0, 1], [2, 3], [4, 5], [6, 7]]

KN = 128 * (QK // 128) * TQ   # bf16 elems of one kT half (128x8x1024)
VN = 128 * (TQ // 128) * QK   # bf16 elems of one v half (128x8x1024)

LAST = None  # BassKernelResults of the most recent run (for test harness)

_cache = {}


def _install_ntff_shim():
    """Provide antenv.axon_hooks + disable artifact upload so that
    run_bass_kernel_spmd(trace=True) can profile under axon in this image."""
    import sys
    import types

    try:
        import antenv.axon_hooks  # noqa: F401
    except ImportError:
        import antenv
        mod = types.ModuleType("antenv.axon_hooks")
        _h = [None]
        mod.set_axon_ntff_profile_hook = lambda h: _h.__setitem__(0, h)
        mod.get_axon_ntff_profile_hook = lambda: _h[0]
        sys.modules["antenv.axon_hooks"] = mod
        antenv.axon_hooks = mod
        try:
            from trn_agent_boot.trn_boot import _ntff_profile_via_ctypes
            mod.set_axon_ntff_profile_hook(
                _ntff_profile_via_ctypes("/opt/axon/libaxon_pjrt.so"))
        except Exception as e:
            print(f"ntff hook registration failed: {e}")
    try:
        import concourse.bass_utils as bu
        bu.upload_artifacts = lambda tmpdir: f"local:{tmpdir}"
    except Exception:
        pass


def _emit(tc, xqT, xkT, xvT, wq, wk, wv, wo, out,
          cc_k_in, cc_k_out, cc_v_in, cc_v_out):
    nc = tc.nc
    exp_f = mybir.ActivationFunctionType.Exp

    with tc.tile_pool(name="persist", bufs=1) as persist:
        kT = persist.tile([128, QK // 128, T], BF, tag="kT")
        vaug = persist.tile([128, T // 128, H, DKH + 1], BF, tag="vaug")
        qT = persist.tile([128, QK // 128, TQ], BF, tag="qT")
        aoT = persist.tile([128, QK // 128, TQ], BF, tag="aoT")
        nc.vector.memset(vaug[:, :, :, DKH:DKH + 1], 1.0)

        # ---- phase 1: own-half k/v projections -> staged to DRAM for the
        # pair AllGathers ----
        with (
            nc.named_scope("p1_kvproj"),
            tc.tile_pool(name="wkv", bufs=1) as wkv_pool,
            tc.tile_pool(name="xk", bufs=16) as xk_pool,
            tc.tile_pool(name="xv", bufs=17) as xv_pool,
            tc.tile_pool(name="stg", bufs=2) as stg_pool,
            tc.tile_pool(name="ps1", bufs=6, space="PSUM") as ps1,
        ):
            wk_sb = wkv_pool.tile([128, KD, QK], BF, tag="wk")
            wv_sb = wkv_pool.tile([128, KD, QK], BF, tag="wv")
            ccin_k = cc_k_in[0, :]
            ccin_v = cc_v_in[0, :]

            # all kT work first so the k AllGather fires at ~half of p1 and
            # flies while the v projection computes
            xv_t = {}

            def load_xv(nb):
                for k in range(KD):
                    xvt = xv_pool.tile([128, 512], BF, tag="xv")
                    nc.sync.dma_start(out=xvt, in_=xvT[k * 128:(k + 1) * 128, nb * 512:(nb + 1) * 512])
                    xv_t[(nb, k)] = xvt

            for nb in range(TQ // 512):  # 2 blocks of own half
                xk_t = []
                for k in range(KD):
                    xkt = xk_pool.tile([128, 512], BF, tag="xk")
                    # interleave weight-chunk and x-chunk loads so the first
                    # matmuls' inputs arrive first
                    if nb == 0:
                        nc.sync.dma_start(out=wk_sb[:, k, :], in_=wk[k * 128:(k + 1) * 128, :])
                        nc.sync.dma_start(out=wv_sb[:, k, :], in_=wv[k * 128:(k + 1) * 128, :])
                    nc.sync.dma_start(out=xkt, in_=xkT[k * 128:(k + 1) * 128, nb * 512:(nb + 1) * 512])
                    xk_t.append(xkt)
                # kT[m-slice, this T block] = wk_slice.T @ xk
                for m in range(QK // 128):  # 8
                    ps = ps1.tile([128, 512], FP32, tag="ps1")
                    for k in range(KD):
                        nc.tensor.matmul(
                            ps, wk_sb[:, k, m * 128:(m + 1) * 128], xk_t[k],
                            start=(k == 0), stop=(k == KD - 1))
                    st = stg_pool.tile([128, 512], BF, tag="stg")
                    nc.vector.tensor_copy(out=st, in_=ps)
                    dst = bass.AP(
                        tensor=ccin_k.tensor,
                        offset=m * 1024 + nb * 512,
                        ap=[[QK // 128 * TQ, 128], [1, 512]])
                    nc.gpsimd.dma_start(out=dst, in_=st)
            load_xv(0)
            nc.gpsimd.collective_compute(
                "AllGather", mybir.AluOpType.bypass,
                replica_groups=GROUPS, ins=[cc_k_in], outs=[cc_k_out])

            # v[T-row slice, V cols] = xv_slice.T @ wv
            for nb in range(TQ // 512):
                if nb == 1:
                    load_xv(1)
                for msl in range(4):
                    ms = nb * 4 + msl
                    for n in range(QK // 512):  # 2
                        ps = ps1.tile([128, 512], FP32, tag="ps1")
                        for k in range(KD):
                            nc.tensor.matmul(
                                ps, xv_t[(nb, k)][:, msl * 128:(msl + 1) * 128],
                                wv_sb[:, k, n * 512:(n + 1) * 512],
                                start=(k == 0), stop=(k == KD - 1))
                        st = stg_pool.tile([128, 512], BF, tag="stg")
                        nc.vector.tensor_copy(out=st, in_=ps)
                        dst = bass.AP(
                            tensor=ccin_v.tensor,
                            offset=ms * 1024 + n * 512,
                            ap=[[TQ // 128 * QK, 128], [1, 512]])
                        nc.gpsimd.dma_start(out=dst, in_=st)
            nc.gpsimd.collective_compute(
                "AllGather", mybir.AluOpType.bypass,
                replica_groups=GROUPS, ins=[cc_v_in], outs=[cc_v_out])

        # ---- phase 2: q projection (covers the AllGathers) + scatter +
        # attention ----
        with (
            nc.named_scope("p2_attn"),
            tc.tile_pool(name="wqp", bufs=1) as wq_pool,
            tc.tile_pool(name="xq", bufs=1) as xq_pool,
            tc.tile_pool(name="pt", bufs=12) as pt_pool,
            tc.tile_pool(name="dv", bufs=1) as dv_pool,
            tc.tile_pool(name="dsc", bufs=4, space="DRAM") as dr_pool,
            tc.tile_pool(name="psq", bufs=2, space="PSUM") as ps_q,
            tc.tile_pool(name="pss", bufs=2, space="PSUM") as ps_s,
            tc.tile_pool(name="pav", bufs=2, space="PSUM") as ps_av,
        ):
            wq_sb = wq_pool.tile([128, KD, QK], BF, tag="wq")
            xq_sb = xq_pool.tile([128, KD, TQ], BF, tag="xq")
            for k in range(KD):
                nc.sync.dma_start(out=wq_sb[:, k, :], in_=wq[k * 128:(k + 1) * 128, :])
                nc.sync.dma_start(out=xq_sb[:, k, :], in_=xqT[k * 128:(k + 1) * 128, :])

            # q projection: PE work that runs while the AllGathers fly
            for g in range(QK // 128):  # 8 head pairs
                for n in range(TQ // 512):  # 2
                    ps = ps_q.tile([128, 512], FP32, tag="psq")
                    for k in range(KD):
                        nc.tensor.matmul(
                            ps, wq_sb[:, k, g * 128:(g + 1) * 128],
                            xq_sb[:, k, n * 512:(n + 1) * 512],
                            start=(k == 0), stop=(k == KD - 1))
                    nc.vector.tensor_copy(out=qT[:, g, n * 512:(n + 1) * 512], in_=ps)

            # scatter the gathered halves (rank order = T order) into kT and
            # vaug; per-group DMAs so early attention unblocks early
            for r in range(2):
                for m in range(QK // 128):
                    src_k = bass.AP(
                        tensor=cc_k_out.tensor,
                        offset=r * KN + m * 1024,
                        ap=[[QK // 128 * TQ, 128], [1, TQ]])
                    nc.sync.dma_start(out=kT[:, m, r * TQ:(r + 1) * TQ], in_=src_k)
                for msl in range(TQ // 128):  # 8
                    ms = r * (TQ // 128) + msl
                    src_v = bass.AP(
                        tensor=cc_v_out.tensor,
                        offset=r * VN + msl * QK,
                        ap=[[TQ // 128 * QK, 128], [DKH, H], [1, DKH]])
                    # gpsimd queue: keeps the 4MB v scatter off the sync
                    # queue that feeds the attention kT reads
                    nc.gpsimd.dma_start(out=vaug[:, ms, :, 0:DKH], in_=src_v)

            NCH = T // 128  # 16 key chunks
            for g in range(QK // 128):  # 8 head pairs
                for tqb in range(TQ // 512):  # 2
                    qs_a = qT[0:64, g, tqb * 512:(tqb + 1) * 512]
                    qs_b = qT[64:128, g, tqb * 512:(tqb + 1) * 512]
                    pav_a = ps_av.tile([DKH + 1, 512], FP32, tag="pav")
                    pav_b = ps_av.tile([DKH + 1, 512], FP32, tag="pav")
                    pts = [None] * NCH
                    for half in range(2):
                        # scores: two row-tiled matmuls per key chunk run
                        # concurrently (head 2g rows 0:64, head 2g+1 rows
                        # 64:128 of the PE array)
                        for ch in range(NCH // 2):
                            c = half * (NCH // 2) + ch
                            pss = ps_s.tile([128, 1024], FP32, tag="pss")
                            nc.tensor.matmul(
                                pss[:, 0:512],
                                kT[0:64, g, c * 128:(c + 1) * 128],
                                qs_a, start=True, stop=True)
                            nc.tensor.matmul(
                                pss[:, 512:1024],
                                kT[64:128, g, c * 128:(c + 1) * 128],
                                qs_b, start=True, stop=True)
                            ptt = pt_pool.tile([128, 1024], BF, tag="pt")
                            nc.scalar.activation(out=ptt, in_=pss, func=exp_f, scale=0.125)
                            pts[c] = ptt
                        # attention-value accumulation for this half block
                        for ch in range(NCH // 2):
                            c = half * (NCH // 2) + ch
                            nc.tensor.matmul(
                                pav_a, vaug[:, c, 2 * g, :],
                                pts[c][:, 0:512],
                                start=(c == 0), stop=(c == NCH - 1))
                            nc.tensor.matmul(
                                pav_b, vaug[:, c, 2 * g + 1, :],
                                pts[c][:, 512:1024],
                                start=(c == 0), stop=(c == NCH - 1))
                    # denominators for both heads -> 1/l -> broadcast.
                    # custom-DVE ops need base-partition-0 operands, so first
                    # copy the l rows (PSUM partition 64) to partition-0 SBUF
                    # tiles with a standard DVE copy, then approx-reciprocal.
                    lr_a = dv_pool.tile([1, 512], FP32, tag="lra")
                    lr_b = dv_pool.tile([1, 512], FP32, tag="lrb")
                    nc.vector.tensor_copy(out=lr_a, in_=pav_a[DKH:DKH + 1, :])
                    nc.vector.tensor_copy(out=lr_b, in_=pav_b[DKH:DKH + 1, :])
                    linv_a = dv_pool.tile([1, 512], FP32, tag="linva")
                    linv_b = dv_pool.tile([1, 512], FP32, tag="linvb")
                    nc.vector.reciprocal_approx_fast(out=linv_a, in_=lr_a)
                    nc.vector.reciprocal_approx_fast(out=linv_b, in_=lr_b)
                    ldr = dr_pool.tile([2, 512], FP32, tag="ldr")
                    nc.gpsimd.dma_start(out=ldr[0:1, :], in_=linv_a)
                    nc.gpsimd.dma_start(out=ldr[1:2, :], in_=linv_b)
                    lbc = dv_pool.tile([128, 512], FP32, tag="lbc")
                    nc.gpsimd.dma_start(
                        out=lbc[0:DKH, :], in_=ldr[0:1, :].to_broadcast([DKH, 512]))
                    nc.gpsimd.dma_start(
                        out=lbc[DKH:128, :], in_=ldr[1:2, :].to_broadcast([DKH, 512]))
                    # pre-copy on DVE so the 2-input mul carries only a
                    # DVE-local wait (TensorTensor ISA allows one wait)
                    lbcc = dv_pool.tile([128, 512], FP32, tag="lbcc")
                    nc.vector.tensor_copy(out=lbcc, in_=lbc)
                    att = dv_pool.tile([128, 512], BF, tag="att")
                    nc.vector.tensor_mul(
                        out=att[0:DKH, :], in0=pav_a[0:DKH, :], in1=lbcc[0:DKH, :])
                    nc.vector.tensor_mul(
                        out=att[DKH:128, :], in0=pav_b[0:DKH, :], in1=lbcc[DKH:128, :])
                    nc.sync.dma_start(
                        out=aoT[:, g, tqb * 512:(tqb + 1) * 512], in_=att)

        # ---- phase 3: output projection ----
        with (
            nc.named_scope("p3_oproj"),
            tc.tile_pool(name="wo", bufs=32) as wo_pool,
            tc.tile_pool(name="ostg", bufs=6) as o_pool,
            tc.tile_pool(name="pso", bufs=6, space="PSUM") as ps_o,
        ):
            KO = QK // 128  # 8
            # all wo tiles upfront: no p2 dependency, so these DMAs land
            # during the attention tail and the first chains never wait
            wo_t = {}
            for nb in range(D // 512):  # 4
                for k in range(KO):
                    wot = wo_pool.tile([128, 512], BF, tag="wo")
                    nc.sync.dma_start(out=wot, in_=wo[k * 128:(k + 1) * 128, nb * 512:(nb + 1) * 512])
                    wo_t[(nb, k)] = wot
            # PE warmth bridge for the p2->p3 transition: reads aoT slice 6
            # (written near the end of attention) so the scheduler places
            # these in the gap before the first real MMs.
            for i in range(12):
                ps = ps_o.tile([128, 512], FP32, tag="pso")
                nc.tensor.matmul(
                    ps, aoT[:, 6, i * 128 % TQ:(i * 128 % TQ) + 128],
                    aoT[:, 6, 0:512], start=True, stop=True)
            for nb in range(D // 512):  # 4
                for m in range(TQ // 128):  # 8
                    ps = ps_o.tile([128, 512], FP32, tag="pso")
                    for k in range(KO):
                        nc.tensor.matmul(
                            ps, aoT[:, k, m * 128:(m + 1) * 128], wo_t[(nb, k)],
                            start=(k == 0), stop=(k == KO - 1))
                    stg = o_pool.tile([128, 512], FP32, tag="ostg")
                    nc.vector.tensor_copy(out=stg, in_=ps)
                    nc.sync.dma_start(
                        out=out[m * 128:(m + 1) * 128, nb * 512:(nb + 1) * 512], in_=stg)


def _build():
    if "nc" in _cache:
        return _cache["nc"]
    nc = bacc.Bacc("TRN2", target_bir_lowering=False, debug=False, num_devices=NCORES)
    xqT = nc.dram_tensor("xqT", [D, TQ], BF, kind="ExternalInput").ap()
    xkT = nc.dram_tensor("xkT", [D, TQ], BF, kind="ExternalInput").ap()
    xvT = nc.dram_tensor("xvT", [D, TQ], BF, kind="ExternalInput").ap()
    wq = nc.dram_tensor("wq", [D, QK], BF, kind="ExternalInput").ap()
    wk = nc.dram_tensor("wk", [D, QK], BF, kind="ExternalInput").ap()
    wv = nc.dram_tensor("wv", [D, QK], BF, kind="ExternalInput").ap()
    wo = nc.dram_tensor("wo", [QK, D], BF, kind="ExternalInput").ap()
    out = nc.dram_tensor("out", [TQ, D], mybir.dt.float32, kind="ExternalOutput").ap()
    cc_k_in = nc.dram_tensor("cc_k_in", [1, KN], BF, kind="Internal").ap()
    cc_k_out = nc.dram_tensor("cc_k_out", [2, KN], BF, kind="Internal").ap()
    cc_v_in = nc.dram_tensor("cc_v_in", [1, VN], BF, kind="Internal").ap()
    cc_v_out = nc.dram_tensor("cc_v_out", [2, VN], BF, kind="Internal").ap()
    with tile.TileContext(nc) as tc:
        _emit(tc, xqT, xkT, xvT, wq, wk, wv, wo, out,
              cc_k_in, cc_k_out, cc_v_in, cc_v_out)
    nc.compile()
    _cache["nc"] = nc
    return nc


def kernel(**inputs):
    global LAST
    Q = np.asarray(inputs["Q"], dtype=np.float32)
    K = np.asarray(inputs["K"], dtype=np.float32)
    V = np.asarray(inputs["V"], dtype=np.float32)
    wq_b = np.asarray(inputs["Wq"], dtype=np.float32).astype(BF16)
    wk_b = np.asarray(inputs["Wk"], dtype=np.float32).astype(BF16)
    wv_b = np.asarray(inputs["Wv"], dtype=np.float32).astype(BF16)
    wo_b = np.asarray(inputs["Wo"], dtype=np.float32).astype(BF16)

    nc = _build()
    in_maps = []
    for core in range(NCORES):
        b, s = core // 2, core % 2
        in_maps.append({
            "xqT": np.ascontiguousarray(Q[b, s * TQ:(s + 1) * TQ, :].T).astype(BF16),
            "xkT": np.ascontiguousarray(K[b, s * TQ:(s + 1) * TQ, :].T).astype(BF16),
            "xvT": np.ascontiguousarray(V[b, s * TQ:(s + 1) * TQ, :].T).astype(BF16),
            "wq": wq_b, "wk": wk_b, "wv": wv_b, "wo": wo_b,
        })
    want_trace = bool(os.environ.get("BASS_TRACE"))
    if want_trace:
        _install_ntff_shim()
        try:
            res = run_bass_kernel_spmd(
                nc, in_maps, core_ids=list(range(NCORES)), trace=True)
        except Exception as e:  # profiling infra missing -> still get results
            print(f"trace run failed ({type(e).__name__}: {e}); retrying untraced")
            res = run_bass_kernel_spmd(nc, in_maps, core_ids=list(range(NCORES)))
    else:
        res = run_bass_kernel_spmd(nc, in_maps, core_ids=list(range(NCORES)))
    LAST = res
    if res.exec_time_ns is not None:
        print(f"HW exec time: {res.exec_time_ns} ns")

    out = np.empty((B, T, D), np.float32)
    for core in range(NCORES):
        b, s = core // 2, core % 2
        out[b, s * TQ:(s + 1) * TQ, :] = res.results[core]["out"]
    return out
